# revision 1
# baseline (speedup 1.0000x reference)
"""Causal single-head attention (S=4096, D=1024, fp32) on 8 TRN2 NeuronCores.

v6 (pair-split proj + chunked pair-AllGather + SBUF-accumulated A@V) with the
serialization fixed: attention quarters are emitted INSIDE the projection
stream (attn t0 between proj q2 and q3, the rest after), so the PE consumes
gathered chunks as they land instead of finishing all projections first.
Projection accumulation and score matmuls share one PSUM pool (4 banks) so
the total PSUM stays at 8: 4 shared + 3 A@V scratch + 1 packed sums.
"""

import numpy as np
import ml_dtypes

import concourse.bacc as bacc
import concourse.tile as tile
from concourse import mybir
from concourse.bass_utils import run_bass_kernel_spmd

S = 4096
D = 1024
NCORES = 8
ROWS = 512
P = 128
DC = 8
OT = 8
HALF = 2048
NQT = 4
NJT = 32
BF = mybir.dt.bfloat16
F32 = mybir.dt.float32
EXP = mybir.ActivationFunctionType.Exp
PAIRS = [[0, 1], [2, 3], [4, 5], [6, 7]]

bf16 = ml_dtypes.bfloat16


def build_nc():
    nc = bacc.Bacc(None, target_bir_lowering=False, debug=False)

    xq = nc.declare_dram_parameter("xqt", [D, ROWS], BF, isOutput=False)
    xk = nc.declare_dram_parameter("xkh", [D, HALF], BF, isOutput=False)
    xv = nc.declare_dram_parameter("xvh", [D, HALF], BF, isOutput=False)
    wq = nc.declare_dram_parameter("wqt", [D, D], BF, isOutput=False)
    wk = nc.declare_dram_parameter("wkt", [D, D], BF, isOutput=False)
    wv = nc.declare_dram_parameter("wvt", [D, D], BF, isOutput=False)
    msk = nc.declare_dram_parameter("mask01", [NJT, P, ROWS], BF, isOutput=False)
    out = nc.declare_dram_parameter("out", [ROWS, D], F32, isOutput=True)

    kvin = [nc.dram_tensor(f"kvin{t}", [P, 16, 512], BF) for t in range(NQT)]
    kvout = [nc.dram_tensor(f"kvout{t}", [2 * P, 16, 512], BF) for t in range(NQT)]


    with tile.TileContext(nc) as tc:
        with (
            tc.tile_pool(name="persist", bufs=1) as persist,
            tc.tile_pool(name="proj", bufs=1) as kp,
            tc.tile_pool(name="stg", bufs=6) as stg,
            tc.tile_pool(name="xs", bufs=2) as xs,
            tc.tile_pool(name="kvs", bufs=2) as kvs,
            tc.tile_pool(name="att", bufs=6) as ap,
            tc.tile_pool(name="att_out", bufs=3) as op,
            tc.tile_pool(name="pps", bufs=5, space="PSUM") as pps,
            tc.tile_pool(name="avs", bufs=2, space="PSUM") as avsum,
            tc.tile_pool(name="ops", bufs=1, space="PSUM") as opsum,
        ):
            ones = persist.tile([P, 16], BF, tag="ones", name="ones")
            nc.vector.memset(ones[:], 1.0)
            zbias = persist.tile([P, 1], F32, tag="zbias", name="zbias")
            nc.vector.memset(zbias[:], 0.0)
            qT = [persist.tile([P, ROWS], BF, tag=f"qT{t}", name=f"qT{t}") for t in range(OT)]
            acc = {}
            for isub in range(4):
                for ob in range(2):
                    acc[isub, ob] = persist.tile([P, 512], F32, tag=f"acc{isub}{ob}", name=f"acc{isub}{ob}")
                    nc.vector.memset(acc[isub, ob][:], 0.0)
            sums_bank = opsum.tile([P, 64], F32, tag="sums", name="sums")

            wk_t = [kp.tile([P, D], BF, tag=f"wk{d_}", name=f"wk{d_}") for d_ in range(DC)]
            wv_t = [kp.tile([P, D], BF, tag=f"wv{d_}", name=f"wv{d_}") for d_ in range(DC)]
            xk_t = {}
            xv_t = {}

            def load_x_quarter(t, k_first=False):
                for d_ in range(DC):
                    xk_t[t, d_] = xs.tile([P, 512], BF, tag=f"xk{d_}", name=f"xk{d_}")
                    nc.sync.dma_start(out=xk_t[t, d_][:], in_=xk[d_ * P:(d_ + 1) * P, t * 512:(t + 1) * 512])
                    if not k_first:
                        xv_t[t, d_] = xs.tile([P, 512], BF, tag=f"xv{d_}", name=f"xv{d_}")
                        nc.sync.dma_start(out=xv_t[t, d_][:], in_=xv[d_ * P:(d_ + 1) * P, t * 512:(t + 1) * 512])
                if k_first:
                    for d_ in range(DC):
                        xv_t[t, d_] = xs.tile([P, 512], BF, tag=f"xv{d_}", name=f"xv{d_}")
                        nc.sync.dma_start(out=xv_t[t, d_][:], in_=xv[d_ * P:(d_ + 1) * P, t * 512:(t + 1) * 512])

            def kv_quarter(t):
                for ohi in range(OT):
                    ps = pps.tile([P, 512], F32, tag="pp", name="ppk")
                    for d_ in range(DC):
                        nc.tensor.matmul(
                            ps[:],
                            lhsT=wk_t[d_][:, ohi * P:(ohi + 1) * P],
                            rhs=xk_t[t, d_][:],
                            start=(d_ == 0),
                            stop=(d_ == DC - 1),
                        )
                    sg = stg.tile([P, 512], BF, tag="sg", name="sg")
                    nc.scalar.copy(sg[:], ps[:])
                    nc.gpsimd.dma_start(out=kvin[t][:, ohi, :], in_=sg[:])
                for jh in range(4):
                    for ob in range(2):
                        ps = pps.tile([P, 512], F32, tag="pp", name="ppv")
                        for d_ in range(DC):
                            nc.tensor.matmul(
                                ps[:],
                                lhsT=xv_t[t, d_][:, jh * P:(jh + 1) * P],
                                rhs=wv_t[d_][:, ob * 512:(ob + 1) * 512],
                                start=(d_ == 0),
                                stop=(d_ == DC - 1),
                            )
                        sg = stg.tile([P, 512], BF, tag="sg", name="sg")
                        nc.scalar.copy(sg[:], ps[:])
                        nc.gpsimd.dma_start(out=kvin[t][:, 8 + ob * 4 + jh, :], in_=sg[:])
                nc.gpsimd.collective_compute(
                    "AllGather",
                    mybir.AluOpType.bypass,
                    replica_groups=PAIRS,
                    ins=[kvin[t][:].opt()],
                    outs=[kvout[t][:].opt()],
                )

            def attn_quarter(qtr):
                t, g = qtr // 2, qtr % 2
                ktq = kvs.tile([P, OT, 512], BF, tag="ktq", name="ktq")
                nc.scalar.dma_start(out=ktq[:], in_=kvout[t][g * P:(g + 1) * P, 0:8, :])
                vtq = kvs.tile([P, OT, 512], BF, tag="vtq", name="vtq")
                nc.scalar.dma_start(out=vtq[:], in_=kvout[t][g * P:(g + 1) * P, 8:16, :])
                ptq = []
                for jl in range(4):
                    jt = qtr * 4 + jl
                    sp = pps.tile([P, ROWS], F32, tag="pp", name="sps")
                    for oc in range(OT):
                        nc.tensor.matmul(
                            sp[:],
                            lhsT=ktq[:, oc, jl * P:(jl + 1) * P],
                            rhs=qT[oc][:],
                            start=(oc == 0),
                            stop=(oc == OT - 1),
                        )
                    pt = ap.tile([P, ROWS], BF, tag="pt", name="pt")
                    nc.scalar.activation(pt[:], sp[:], EXP, bias=zbias[:])
                    mt = ap.tile([P, ROWS], BF, tag="mt", name="mt")
                    nc.gpsimd.dma_start(out=mt[:], in_=msk[jt, :, :])
                    nc.vector.tensor_mul(pt[:], pt[:], mt[:])
                    ptq.append(pt)
                    for isub in range(4):
                        nc.tensor.matmul(
                            sums_bank[:, isub * 16:(isub + 1) * 16],
                            lhsT=pt[:, isub * P:(isub + 1) * P],
                            rhs=ones[:],
                            start=(jt == 0 and isub == 0),
                            stop=(jt == NJT - 1 and isub == 3),
                            skip_group_check=True,
                        )
                for isub in range(4):
                    for ob in range(2):
                        sc = avsum.tile([P, 512], F32, tag="avs", name="avs")
                        for jl in range(4):
                            nc.tensor.matmul(
                                sc[:],
                                lhsT=ptq[jl][:, isub * P:(isub + 1) * P],
                                rhs=vtq[:, ob * 4 + jl, :],
                                start=(jl == 0),
                                stop=(jl == 3),
                            )
                        nc.vector.tensor_add(acc[isub, ob][:], acc[isub, ob][:], sc[:])

            # ---- interleaved schedule: K weights + x_k q0 load first ----
            for d_ in range(DC):
                nc.sync.dma_start(out=wk_t[d_][:], in_=wk[d_ * P:(d_ + 1) * P, :])
            load_x_quarter(0, k_first=True)
            for d_ in range(DC):
                nc.sync.dma_start(out=wv_t[d_][:], in_=wv[d_ * P:(d_ + 1) * P, :])
            kv_quarter(0)
            load_x_quarter(1)
            kv_quarter(1)

            xq_t = [kp.tile([P, ROWS], BF, tag=f"xq{d_}", name=f"xq{d_}") for d_ in range(DC)]
            wq_t = [kp.tile([P, D], BF, tag=f"wq{d_}", name=f"wq{d_}") for d_ in range(DC)]
            for d_ in range(DC):
                nc.sync.dma_start(out=xq_t[d_][:], in_=xq[d_ * P:(d_ + 1) * P, :])
                nc.sync.dma_start(out=wq_t[d_][:], in_=wq[d_ * P:(d_ + 1) * P, :])
            for t in range(OT):
                ps = pps.tile([P, ROWS], F32, tag="pp", name="ppq")
                for d_ in range(DC):
                    nc.tensor.matmul(
                        ps[:],
                        lhsT=wq_t[d_][:, t * P:(t + 1) * P],
                        rhs=xq_t[d_][:],
                        start=(d_ == 0),
                        stop=(d_ == DC - 1),
                    )
                nc.vector.tensor_copy(qT[t][:], ps[:])

            load_x_quarter(2)
            kv_quarter(2)
            attn_quarter(0)
            load_x_quarter(3)
            kv_quarter(3)
            for qtr in range(1, 8):
                attn_quarter(qtr)

            for isub in range(4):
                ssb = op.tile([P, 1], F32, tag="ssb", name="ssb")
                nc.vector.tensor_copy(ssb[:], sums_bank[:, isub * 16:isub * 16 + 1])
                rec = op.tile([P, 1], F32, tag=f"rec{isub}", name=f"rec{isub}")
                nc.vector.reciprocal(rec[:], ssb[:])
                for ob in range(2):
                    osb = op.tile([P, 512], F32, tag="osb", name="osb")
                    nc.vector.tensor_scalar_mul(osb[:], acc[isub, ob][:], rec[:])
                    nc.sync.dma_start(out=out[isub * P:(isub + 1) * P, ob * 512:(ob + 1) * 512], in_=osb[:])
    return nc


_CACHE = {}


def _get_nc():
    if "nc" not in _CACHE:
        nc = build_nc()
        nc.compile()
        _CACHE["nc"] = nc
    return _CACHE["nc"]


def build_in_maps(inputs):
    x_q = np.asarray(inputs["encodings_for_q"], dtype=np.float32)
    x_k = np.asarray(inputs["encodings_for_k"], dtype=np.float32)
    x_v = np.asarray(inputs["encodings_for_v"], dtype=np.float32)
    W_q = np.asarray(inputs["W_q"], dtype=np.float32)
    W_k = np.asarray(inputs["W_k"], dtype=np.float32)
    W_v = np.asarray(inputs["W_v"], dtype=np.float32)

    wqt = np.ascontiguousarray(W_q.T).astype(bf16)
    wkt = np.ascontiguousarray(W_k.T / np.sqrt(D)).astype(bf16)
    wvt = np.ascontiguousarray(W_v.T).astype(bf16)

    causal = (np.arange(S)[:, None] <= np.arange(S)[None, :])

    in_maps = []
    for c in range(NCORES):
        rows = slice(ROWS * c, ROWS * (c + 1))
        h = slice(HALF * (c % 2), HALF * (c % 2 + 1))
        xqt_c = np.ascontiguousarray(x_q[rows].T).astype(bf16)
        xkh_c = np.ascontiguousarray(x_k[h].T).astype(bf16)
        xvh_c = np.ascontiguousarray(x_v[h].T).astype(bf16)
        m = causal[:, rows]
        mg = m.reshape(NJT, P, ROWS)
        order = []
        for qtr in range(8):
            t, g = qtr // 2, qtr % 2
            for jl in range(4):
                order.append(16 * g + 4 * t + jl)
        mask_c = np.ascontiguousarray(mg[order]).astype(bf16)
        in_maps.append(
            dict(
                xqt=xqt_c, xkh=xkh_c, xvh=xvh_c,
                wqt=wqt, wkt=wkt, wvt=wvt,
                mask01=mask_c,
            )
        )
    return in_maps


def kernel(**inputs):
    nc = _get_nc()
    in_maps = build_in_maps(inputs)
    res = run_bass_kernel_spmd(nc, in_maps, list(range(NCORES)))
    outs = [np.asarray(res.results[i]["out"], dtype=np.float32) for i in range(NCORES)]
    return np.concatenate(outs, axis=0)



# revision 11
# speedup vs baseline: 1.3922x; 1.3922x over previous
"""Causal single-head attention (S=4096, D=1024, fp32) on 8 TRN2 NeuronCores.

v7: uniform interleaved-row causal scheme.
- Core c owns q rows c::8 (mod-8 interleave): its 4 q-tiles of 128 rows span
  global ranges [1024t, 1024(t+1)) and need key-tiles tau < 8(t+1) -- the SAME
  structure on every core (SPMD-uniform), yet ~half the score/AV work of the
  full rectangle is skipped by causality.
- K/V projections are sharded 8-way (rank r projects key-tiles {r, r+8, r+16,
  r+24}) and shared via chip-wide AllGathers: K as fp8 (4MB out), V as two
  bf16 halves (4MB out each) so A@V vd-half-0 can start before half-1 lands.
  A tiny dummy AG first absorbs the ncfw warmup + rank-skew barrier.
- Scores run fp8 DoubleRow (2x contraction per matmul); exp folds the 1/sqrt(D)
  scale; A@V and projections stay bf16 (fp8 there fails the 2e-2 gate).
- Diagonal masks are 8 host-provided [128,128] 0/1 bf16 tiles per core
  (mask_d[ik,iq] = 128d + ik <= 8*iq + c), applied to the first 128 q-cols of
  every score tile.
"""

import numpy as np
import ml_dtypes

import concourse.bacc as bacc
import concourse.tile as tile
from concourse import mybir
from concourse.bass_utils import run_bass_kernel_spmd

S = 4096
D = 1024
NCORES = 8
P = 128
DC = 8          # contraction blocks of 128 in D
NT = 4          # q-tiles per core (128 rows each)
NKT = 32        # key tiles of 128 globally
BF = mybir.dt.bfloat16
F32 = mybir.dt.float32
FP8 = mybir.dt.float8e4
EXP = mybir.ActivationFunctionType.Exp
DR = mybir.MatmulPerfMode.DoubleRow
ALL8 = [[0, 1, 2, 3, 4, 5, 6, 7]]

bf16 = ml_dtypes.bfloat16
f8 = ml_dtypes.float8_e4m3


def ntiles(tau):
    """number of q-tiles that need key-tile tau (t >= tau//8)"""
    return NT - tau // 8


def build_nc(dbg=False):
    nc = bacc.Bacc(None, target_bir_lowering=False, debug=False)
    if dbg:
        d_q = nc.declare_dram_parameter("d_q", [P, DC, 512], FP8, isOutput=True)
        d_k0 = nc.declare_dram_parameter("d_k0", [P, DC, P], FP8, isOutput=True)
        d_pt = nc.declare_dram_parameter("d_pt", [4, P, 512], BF, isOutput=True)
        d_sums = nc.declare_dram_parameter("d_sums", [P, 64], F32, isOutput=True)

    xq = nc.declare_dram_parameter("xqT", [D, 512], BF, isOutput=False)
    xk = nc.declare_dram_parameter("xkT", [D, 512], BF, isOutput=False)
    xv = nc.declare_dram_parameter("xvT", [D, 512], BF, isOutput=False)
    wq = nc.declare_dram_parameter("wqT", [D, D], BF, isOutput=False)
    wk = nc.declare_dram_parameter("wkT", [D, D], BF, isOutput=False)
    wv = nc.declare_dram_parameter("wvT", [D, D], BF, isOutput=False)
    msk = nc.declare_dram_parameter("masks", [8, P, P], BF, isOutput=False)
    out = nc.declare_dram_parameter("out", [512, D], F32, isOutput=True)

    # collective buffers (internal DRAM)
    agw_i = nc.dram_tensor("agw_i", [P, 16], BF)
    agw_o = nc.dram_tensor("agw_o", [NCORES * P, 16], BF, addr_space="Shared")
    kvin_k = nc.dram_tensor("kvin_k", [P, NT, DC, P], FP8)
    kvout_k = nc.dram_tensor("kvout_k", [NCORES * P, NT, DC, P], FP8, addr_space="Shared")
    kvin_v = [nc.dram_tensor(f"kvin_v{h}", [P, NT, 512], BF) for h in range(2)]
    kvout_v = [nc.dram_tensor(f"kvout_v{h}", [NCORES * P, NT, 512], BF, addr_space="Shared")
               for h in range(2)]

    with tile.TileContext(nc) as tc:
        with (
            tc.tile_pool(name="persist", bufs=1) as persist,
            tc.tile_pool(name="wp", bufs=16) as wp,
            tc.tile_pool(name="xp", bufs=16) as xp,
            tc.tile_pool(name="ptp", bufs=1) as ptp,
            tc.tile_pool(name="ktp", bufs=3) as ktp,
            tc.tile_pool(name="vtp", bufs=4) as vtp,
            tc.tile_pool(name="outp", bufs=4) as outp,
            tc.tile_pool(name="sps", bufs=3, space="PSUM") as sps,
            tc.tile_pool(name="avs", bufs=1, space="PSUM") as avs,
            tc.tile_pool(name="sums", bufs=1, space="PSUM") as sums_pool,
        ):
            ones = persist.tile([P, 16], BF, tag="ones", name="ones")
            nc.vector.memset(ones[:], 1.0)
            agw_s = persist.tile([P, 16], BF, tag="agw", name="agw")
            nc.vector.memset(agw_s[:], 0.0)

            # warmup collective: absorbs launch barrier + ncfw first-call cost
            nc.sync.dma_start(out=agw_i[:], in_=agw_s[:])
            nc.gpsimd.collective_compute(
                "AllGather", mybir.AluOpType.bypass, replica_groups=ALL8,
                ins=[agw_i[:].opt()], outs=[agw_o[:].opt()],
            )

            # masks: 8 [128,128] bf16 tiles
            m_t = [persist.tile([P, P], BF, tag=f"m{d}", name=f"m{d}") for d in range(8)]
            for d in range(8):
                nc.gpsimd.dma_start(out=m_t[d][:], in_=msk[d, :, :])

            qT = persist.tile([P, DC, 512], FP8, tag="qT", name="qT")
            kloc = persist.tile([P, NT, DC, P], FP8, tag="kloc", name="kloc")
            vloc = [persist.tile([P, NT, 512], BF, tag=f"vloc{h}", name=f"vloc{h}")
                    for h in range(2)]

            # ---- K projection: kT blocks [outdim 128, 512 keys] ----
            wk_t = [wp.tile([P, D], BF, tag="w", name=f"wk{d}") for d in range(DC)]
            xk_t = [xp.tile([P, 512], BF, tag="x", name=f"xk{d}") for d in range(DC)]
            for d in range(DC):
                nc.sync.dma_start(out=xk_t[d][:], in_=xk[d * P:(d + 1) * P, :])
                nc.sync.dma_start(out=wk_t[d][:], in_=wk[d * P:(d + 1) * P, :])
            for ob in range(DC):
                ps = sps.tile([P, 512], F32, tag="sp", name="ppk")
                for d in range(DC):
                    nc.tensor.matmul(
                        ps[:], lhsT=wk_t[d][:, ob * P:(ob + 1) * P], rhs=xk_t[d][:],
                        start=(d == 0), stop=(d == DC - 1),
                    )
                # quantize to fp8, scatter keys into slot-major layout
                for s in range(NT):
                    nc.scalar.copy(kloc[:, s, ob, :], ps[:, s * P:(s + 1) * P])
            nc.sync.dma_start(out=kvin_k[:], in_=kloc[:])
            nc.gpsimd.collective_compute(
                "AllGather", mybir.AluOpType.bypass, replica_groups=ALL8,
                ins=[kvin_k[:].opt()], outs=[kvout_k[:].opt()],
            )

            # ---- V projection: v blocks [keys 128, 512 vd] ----
            wv_t = [wp.tile([P, D], BF, tag="w", name=f"wv{d}") for d in range(DC)]
            xv_t = [xp.tile([P, 512], BF, tag="x", name=f"xv{d}") for d in range(DC)]
            for d in range(DC):
                nc.sync.dma_start(out=xv_t[d][:], in_=xv[d * P:(d + 1) * P, :])
                nc.sync.dma_start(out=wv_t[d][:], in_=wv[d * P:(d + 1) * P, :])
            for h in range(2):
                for s in range(NT):
                    ps = sps.tile([P, 512], F32, tag="sp", name="ppv")
                    for d in range(DC):
                        nc.tensor.matmul(
                            ps[:], lhsT=xv_t[d][:, s * P:(s + 1) * P],
                            rhs=wv_t[d][:, h * 512:(h + 1) * 512],
                            start=(d == 0), stop=(d == DC - 1),
                        )
                    nc.scalar.copy(vloc[h][:, s, :], ps[:])
                nc.sync.dma_start(out=kvin_v[h][:], in_=vloc[h][:])
                nc.gpsimd.collective_compute(
                    "AllGather", mybir.AluOpType.bypass, replica_groups=ALL8,
                    ins=[kvin_v[h][:].opt()], outs=[kvout_v[h][:].opt()],
                )

            # ---- Q projection -> fp8 qT [128, cb, 512] ----
            wq_t = [wp.tile([P, D], BF, tag="w", name=f"wq{d}") for d in range(DC)]
            xq_t = [xp.tile([P, 512], BF, tag="x", name=f"xq{d}") for d in range(DC)]
            for d in range(DC):
                nc.sync.dma_start(out=xq_t[d][:], in_=xq[d * P:(d + 1) * P, :])
                nc.sync.dma_start(out=wq_t[d][:], in_=wq[d * P:(d + 1) * P, :])
            for ob in range(DC):
                ps = sps.tile([P, 512], F32, tag="sp", name="ppq")
                for d in range(DC):
                    nc.tensor.matmul(
                        ps[:], lhsT=wq_t[d][:, ob * P:(ob + 1) * P], rhs=xq_t[d][:],
                        start=(d == 0), stop=(d == DC - 1),
                    )
                nc.scalar.copy(qT[:, ob, :], ps[:])

            # ---- scores sweep: tau = 0..31, fp8 DoubleRow (kt prefetch depth 2) ----
            pt = {}
            kt_t = {}
            for tau in range(NKT + 2):
                if tau < NKT:
                    r, sl = tau % NCORES, tau // NCORES
                    kt = ktp.tile([P, DC, P], FP8, tag="kt", name="kt")
                    kt_t[tau] = kt
                    nc.sync.dma_start(
                        out=kt[:], in_=kvout_k[r * P:(r + 1) * P, sl, :, :])
                if tau < 2:
                    continue
                tc_ = tau - 2
                tmin = tc_ // 8
                N = (NT - tmin) * P
                ps = sps.tile([P, 512], F32, tag="sp", name="sps")
                for dd in range(4):
                    nc.tensor.matmul(
                        ps[:, :N], lhsT=kt_t[tc_][:, 2 * dd:2 * dd + 2, :],
                        rhs=qT[:, 2 * dd:2 * dd + 2, tmin * P:512],
                        start=(dd == 0), stop=(dd == 3), perf_mode=DR,
                    )
                p = ptp.tile([P, 512], BF, tag=f"pt{tc_}", name=f"pt{tc_}")
                pt[tc_] = p
                nc.scalar.activation(p[:, :N], ps[:, :N], EXP, scale=0.03125)
                nc.vector.tensor_mul(p[:, 0:P], p[:, 0:P], m_t[tc_ % 8][:])
                if dbg and tc_ < 4:
                    nc.gpsimd.dma_start(out=d_pt[tc_, :, :], in_=p[:])
                if dbg and tc_ == 0:
                    nc.gpsimd.dma_start(out=d_k0[:], in_=kt_t[0][:])
                    nc.gpsimd.dma_start(out=d_q[:], in_=qT[:])

            # ---- A@V + sums, vd-half 0 sweep, then vd-half 1 sweep ----
            sums_bank = sums_pool.tile([P, 64], F32, tag="sums", name="sums")
            av = {}
            rec = {}
            for h in range(2):
                for t in range(NT):
                    av[t] = avs.tile([P, 512], F32, tag=f"av{t}", name=f"av{t}")
                for tau in range(NKT):
                    r, sl = tau % NCORES, tau // NCORES
                    tmin = tau // 8
                    vt = vtp.tile([P, 512], BF, tag="vt", name="vt")
                    nc.scalar.dma_start(
                        out=vt[:], in_=kvout_v[h][r * P:(r + 1) * P, sl, :])
                    for t in range(tmin, NT):
                        pslice = pt[tau][:, (t - tmin) * P:(t - tmin + 1) * P]
                        if h == 0:
                            # start=True clears has_written for the WHOLE bank:
                            # only the first MM into this bank may set it.
                            nc.tensor.matmul(
                                sums_bank[:, t * 16:(t + 1) * 16],
                                lhsT=pslice, rhs=ones[:],
                                start=(tau == 0 and t == 0),
                                stop=(tau == NKT - 1 and t == NT - 1),
                                skip_group_check=True,
                            )
                        nc.tensor.matmul(
                            av[t][:], lhsT=pslice, rhs=vt[:],
                            start=(tau == 0), stop=(tau == 8 * t + 7),
                            skip_group_check=True,
                        )
                    # close out finished q-tiles
                    if tau % 8 == 7:
                        t = tau // 8
                        if h == 0:
                            if dbg and tau == NKT - 1:
                                dsb = outp.tile([P, 64], F32, tag="dsb", name="dsb")
                                nc.vector.tensor_copy(dsb[:], sums_bank[:])
                                nc.gpsimd.dma_start(out=d_sums[:], in_=dsb[:])
                            rc = outp.tile([P, 1], F32, tag=f"rec{t}", name=f"rec{t}")
                            rec[t] = rc
                            nc.vector.reciprocal(rc[:], sums_bank[:, t * 16:t * 16 + 1])
                        ot = outp.tile([P, 512], F32, tag="ot", name="ot")
                        nc.vector.tensor_scalar_mul(ot[:], av[t][:], rec[t][:])
                        nc.sync.dma_start(
                            out=out[t * P:(t + 1) * P, h * 512:(h + 1) * 512],
                            in_=ot[:])
    return nc


_CACHE = {}


def _get_nc():
    if "nc" not in _CACHE:
        nc = build_nc()
        nc.compile()
        _CACHE["nc"] = nc
    return _CACHE["nc"]


def build_in_maps(inputs):
    x_q = np.asarray(inputs["encodings_for_q"], dtype=np.float32)
    x_k = np.asarray(inputs["encodings_for_k"], dtype=np.float32)
    x_v = np.asarray(inputs["encodings_for_v"], dtype=np.float32)
    W_q = np.asarray(inputs["W_q"], dtype=np.float32)
    W_k = np.asarray(inputs["W_k"], dtype=np.float32)
    W_v = np.asarray(inputs["W_v"], dtype=np.float32)

    # no 1/sqrt(D) folding: the exp activation applies scale=1/32
    wqT = np.ascontiguousarray(W_q.T).astype(bf16)
    wkT = np.ascontiguousarray(W_k.T).astype(bf16)
    wvT = np.ascontiguousarray(W_v.T).astype(bf16)

    ik = np.arange(P)[:, None]
    iq = np.arange(P)[None, :]

    in_maps = []
    for c in range(NCORES):
        rows = np.arange(c, S, NCORES)               # q rows, 512 of them
        keys = np.concatenate([np.arange(P * (8 * s + c), P * (8 * s + c) + P)
                               for s in range(NT)])  # key tiles {c, c+8, c+16, c+24}
        masks = np.stack([(128 * d + ik <= 8 * iq + c) for d in range(8)])
        in_maps.append(dict(
            xqT=np.ascontiguousarray(x_q[rows].T).astype(bf16),
            xkT=np.ascontiguousarray(x_k[keys].T).astype(bf16),
            xvT=np.ascontiguousarray(x_v[keys].T).astype(bf16),
            wqT=wqT, wkT=wkT, wvT=wvT,
            masks=masks.astype(bf16),
        ))
    return in_maps


def kernel(**inputs):
    nc = _get_nc()
    in_maps = build_in_maps(inputs)
    res = run_bass_kernel_spmd(nc, in_maps, list(range(NCORES)))
    full = np.empty((S, D), dtype=np.float32)
    for c in range(NCORES):
        full[c::NCORES] = np.asarray(res.results[c]["out"], dtype=np.float32)
    return full


# revision 13
# speedup vs baseline: 1.6802x; 1.2069x over previous
"""Causal single-head attention (S=4096, D=1024, fp32) on 8 TRN2 NeuronCores.

v8: uniform interleaved-row causal scheme, rank-major streaming.
- Core c owns q rows c::8: its 4 q-tiles of 128 rows span global ranges
  [1024t, 1024(t+1)) and need key-tiles tau < 8(t+1) -- identical structure on
  every core (SPMD-uniform), yet ~half the score/AV work of the full rectangle
  is skipped by causality.
- K/V projections sharded 8-way (rank r projects key-tiles {r,r+8,r+16,r+24});
  shared via 3 chip-wide AllGathers (K fp8, V bf16 in two vd-halves). The ~52us
  launch barrier is runtime-fixed, so AGs execute no earlier than ~58us
  regardless of trigger time; the pre-AG window is filled with q-projection and
  a LOCAL attention pass on the core's own 4 key-tiles (kloc/vloc, no AG dep).
  Own-rank stream tiles are zeroed via a per-tile exp bias input (-1e9), which
  keeps the stream loop uniform across cores.
- Gathered K/V are streamed per RANK-block (512KB contiguous DMAs, 4KB lines)
  to avoid 1KB-packet DMA crawl.
- Scores in fp8 DoubleRow; exp folds the 1/sqrt(D) scale; projections and A@V
  stay bf16 (fp8 there breaks the 2e-2 gate).
"""

import numpy as np
import ml_dtypes

import concourse.bacc as bacc
import concourse.tile as tile
from concourse import mybir
from concourse.bass_utils import run_bass_kernel_spmd

S = 4096
D = 1024
NCORES = 8
P = 128
DC = 8          # contraction blocks of 128 in D
NT = 4          # q-tiles per core (128 rows each)
NKT = 32        # key tiles of 128 globally
BF = mybir.dt.bfloat16
F32 = mybir.dt.float32
FP8 = mybir.dt.float8e4
EXP = mybir.ActivationFunctionType.Exp
DR = mybir.MatmulPerfMode.DoubleRow
ALL8 = [[0, 1, 2, 3, 4, 5, 6, 7]]

bf16 = ml_dtypes.bfloat16


def build_nc(dbg=False):
    nc = bacc.Bacc(None, target_bir_lowering=False, debug=False)
    if dbg:
        d_pt = nc.declare_dram_parameter("d_pt", [4, P, 512], BF, isOutput=True)
        d_sums = nc.declare_dram_parameter("d_sums", [P, 64], F32, isOutput=True)

    xq = nc.declare_dram_parameter("xqT", [D, 512], BF, isOutput=False)
    xk = nc.declare_dram_parameter("xkT", [D, 512], BF, isOutput=False)
    xv = nc.declare_dram_parameter("xvT", [D, 512], BF, isOutput=False)
    wq = nc.declare_dram_parameter("wqT", [D, D], BF, isOutput=False)
    wk = nc.declare_dram_parameter("wkT", [D, D], BF, isOutput=False)
    wv = nc.declare_dram_parameter("wvT", [D, D], BF, isOutput=False)
    msk = nc.declare_dram_parameter("masks", [9, P, P], BF, isOutput=False)
    bias_p = nc.declare_dram_parameter("biasv", [P, NKT], F32, isOutput=False)
    out = nc.declare_dram_parameter("out", [512, D], F32, isOutput=True)

    kvin_k = nc.dram_tensor("kvin_k", [P, NT, DC, P], FP8)
    kvout_k = nc.dram_tensor("kvout_k", [NCORES * P, NT, DC, P], FP8, addr_space="Shared")
    kvin_v = [nc.dram_tensor(f"kvin_v{h}", [P, NT, 512], BF) for h in range(2)]
    kvout_v = [nc.dram_tensor(f"kvout_v{h}", [NCORES * P, NT, 512], BF, addr_space="Shared")
               for h in range(2)]

    with tile.TileContext(nc) as tc:
        with (
            tc.tile_pool(name="persist", bufs=1) as persist,
            tc.tile_pool(name="wp", bufs=16) as wp,
            tc.tile_pool(name="xp", bufs=16) as xp,
            tc.tile_pool(name="ptp", bufs=1) as ptp,
            tc.tile_pool(name="ktp", bufs=2) as ktp,
            tc.tile_pool(name="vtp", bufs=3) as vtp,
            tc.tile_pool(name="outp", bufs=4) as outp,
            tc.tile_pool(name="sps", bufs=3, space="PSUM") as sps,
            tc.tile_pool(name="avs", bufs=1, space="PSUM") as avs,
            tc.tile_pool(name="sums", bufs=1, space="PSUM") as sums_pool,
        ):
            ones = persist.tile([P, 16], BF, tag="ones", name="ones")
            nc.vector.memset(ones[:], 1.0)

            m_t = [persist.tile([P, P], BF, tag=f"m{d}", name=f"m{d}") for d in range(9)]
            bias_t = persist.tile([P, NKT], F32, tag="bias", name="bias")

            qT = persist.tile([P, DC, 512], FP8, tag="qT", name="qT")
            kloc = persist.tile([P, NT, DC, P], FP8, tag="kloc", name="kloc")
            vloc = [persist.tile([P, NT, 512], BF, tag=f"vloc{h}", name=f"vloc{h}")
                    for h in range(2)]

            # ---- K projection: kT blocks [outdim 128, 512 keys] -> fp8 ----
            wk_t = [wp.tile([P, D], BF, tag="w", name=f"wk{d}") for d in range(DC)]
            xk_t = [xp.tile([P, 512], BF, tag="x", name=f"xk{d}") for d in range(DC)]
            for d in range(DC):
                nc.sync.dma_start(out=xk_t[d][:], in_=xk[d * P:(d + 1) * P, :])
                nc.sync.dma_start(out=wk_t[d][:], in_=wk[d * P:(d + 1) * P, :])
            for ob in range(DC):
                ps = sps.tile([P, 512], F32, tag="sp", name="ppk")
                for d in range(DC):
                    nc.tensor.matmul(
                        ps[:], lhsT=wk_t[d][:, ob * P:(ob + 1) * P], rhs=xk_t[d][:],
                        start=(d == 0), stop=(d == DC - 1),
                    )
                for s in range(NT):
                    nc.scalar.copy(kloc[:, s, ob, :], ps[:, s * P:(s + 1) * P])

            # ---- V projection: v blocks [keys 128, 512 vd] -> bf16 ----
            wv_t = [wp.tile([P, D], BF, tag="w", name=f"wv{d}") for d in range(DC)]
            xv_t = [xp.tile([P, 512], BF, tag="x", name=f"xv{d}") for d in range(DC)]
            for d in range(DC):
                nc.sync.dma_start(out=xv_t[d][:], in_=xv[d * P:(d + 1) * P, :])
                nc.sync.dma_start(out=wv_t[d][:], in_=wv[d * P:(d + 1) * P, :])
            for h in range(2):
                for s in range(NT):
                    ps = sps.tile([P, 512], F32, tag="sp", name="ppv")
                    for d in range(DC):
                        nc.tensor.matmul(
                            ps[:], lhsT=xv_t[d][:, s * P:(s + 1) * P],
                            rhs=wv_t[d][:, h * 512:(h + 1) * 512],
                            start=(d == 0), stop=(d == DC - 1),
                        )
                    nc.scalar.copy(vloc[h][:, s, :], ps[:])

            # ---- Q projection -> fp8 qT [128, cb, 512] ----
            wq_t = [wp.tile([P, D], BF, tag="w", name=f"wq{d}") for d in range(DC)]
            xq_t = [xp.tile([P, 512], BF, tag="x", name=f"xq{d}") for d in range(DC)]
            for d in range(DC):
                nc.sync.dma_start(out=xq_t[d][:], in_=xq[d * P:(d + 1) * P, :])
                nc.sync.dma_start(out=wq_t[d][:], in_=wq[d * P:(d + 1) * P, :])
            # constants arriving on sync after the big loads
            for dd in range(9):
                nc.sync.dma_start(out=m_t[dd][:], in_=msk[dd, :, :])
            nc.sync.dma_start(out=bias_t[:], in_=bias_p[:])
            for ob in range(DC):
                ps = sps.tile([P, 512], F32, tag="sp", name="ppq")
                for d in range(DC):
                    nc.tensor.matmul(
                        ps[:], lhsT=wq_t[d][:, ob * P:(ob + 1) * P], rhs=xq_t[d][:],
                        start=(d == 0), stop=(d == DC - 1),
                    )
                nc.scalar.copy(qT[:, ob, :], ps[:])

            # ---- collectives: triggers queue before the barrier ends ----
            nc.sync.dma_start(out=kvin_k[:], in_=kloc[:])
            nc.gpsimd.collective_compute(
                "AllGather", mybir.AluOpType.bypass, replica_groups=ALL8,
                ins=[kvin_k[:].opt()], outs=[kvout_k[:].opt()],
            )
            for h in range(2):
                nc.sync.dma_start(out=kvin_v[h][:], in_=vloc[h][:])
                nc.gpsimd.collective_compute(
                    "AllGather", mybir.AluOpType.bypass, replica_groups=ALL8,
                    ins=[kvin_v[h][:].opt()], outs=[kvout_v[h][:].opt()],
                )

            sums_bank = sums_pool.tile([P, 64], F32, tag="sums", name="sums")
            av = {}
            for t in range(NT):
                av[t] = avs.tile([P, 512], F32, tag=f"av{t}", name=f"av{t}")

            def scores_tile(kt_ap, rhs_hi, tmin, mask, bias_ap, ptag):
                """DR scores + exp + diag mask for one key tile; returns pt."""
                N = rhs_hi - tmin * P
                ps = sps.tile([P, 512], F32, tag="sp", name="sps")
                for dd in range(4):
                    nc.tensor.matmul(
                        ps[:, :N], lhsT=kt_ap[:, 2 * dd:2 * dd + 2, :],
                        rhs=qT[:, 2 * dd:2 * dd + 2, tmin * P:rhs_hi],
                        start=(dd == 0), stop=(dd == 3), perf_mode=DR,
                    )
                p = ptp.tile([P, 512], BF, tag=ptag, name=ptag)
                if bias_ap is None:
                    nc.scalar.activation(p[:, :N], ps[:, :N], EXP, scale=0.03125)
                else:
                    nc.scalar.activation(p[:, :N], ps[:, :N], EXP, scale=0.03125,
                                         bias=bias_ap)
                nc.vector.tensor_mul(p[:, 0:P], p[:, 0:P], mask[:])
                return p

            def sums_av(p, s, t, vt_ap, first, last, do_sums):
                """piggybacked sums + one AV MM for pair (s, t)"""
                pslice = p[:, (t - s) * P:(t - s + 1) * P]
                if do_sums:
                    nc.tensor.matmul(
                        sums_bank[:, t * 16:(t + 1) * 16], lhsT=pslice, rhs=ones[:],
                        start=first and t == 0, stop=last and t == NT - 1,
                        skip_group_check=True,
                    )
                nc.tensor.matmul(
                    av[t][:], lhsT=pslice, rhs=vt_ap,
                    start=first, stop=last, skip_group_check=True,
                )

            # ---- local pass: own 4 key-tiles, no AG dependency ----
            ptl = {}
            for s in range(NT):
                ptl[s] = scores_tile(kloc[:, s, :, :], 512, s, m_t[8], None, f"ptl{s}")
                if dbg and s < 2:
                    nc.gpsimd.dma_start(out=d_pt[s, :, :], in_=ptl[s][:])
                for t in range(s, NT):
                    sums_av(ptl[s], s, t, vloc[0][:, s, :], first=(s == 0), last=False,
                            do_sums=True)

            # ---- stream scores, rank-major (own rank zeroed via bias) ----
            pt = {}
            ktb = {}
            for r in range(NCORES + 1):
                if r < NCORES:
                    kb = ktp.tile([P, NT, DC, P], FP8, tag="ktb", name="ktb")
                    ktb[r] = kb
                    nc.sync.dma_start(out=kb[:], in_=kvout_k[r * P:(r + 1) * P, :, :, :])
                if r < 1:
                    continue
                rr = r - 1
                for s in range(NT):
                    tau = 8 * s + rr
                    pt[tau] = scores_tile(
                        ktb[rr][:, s, :, :], 512, s, m_t[rr],
                        bias_t[:, tau:tau + 1], f"pt{tau}")

            # ---- A@V vd-half sweeps, rank-major ----
            for h in range(2):
                if h == 1:
                    # re-alloc accumulators (banks freed after h=0 normalize)
                    for t in range(NT):
                        av[t] = avs.tile([P, 512], F32, tag=f"av{t}", name=f"av{t}")
                    # local pairs first: vloc-only, runs during AG_v1 wait
                    for s in range(NT):
                        for t in range(s, NT):
                            sums_av(ptl[s], s, t, vloc[1][:, s, :], first=(s == 0),
                                    last=False, do_sums=False)
                for r in range(NCORES):
                    vb = vtp.tile([P, NT, 512], BF, tag="vtb", name="vtb")
                    nc.scalar.dma_start(
                        out=vb[:], in_=kvout_v[h][r * P:(r + 1) * P, :, :])
                    for s in range(NT):
                        tau = 8 * s + r
                        for t in range(s, NT):
                            sums_av(pt[tau], s, t, vb[:, s, :], first=False,
                                    last=(r == NCORES - 1 and s == NT - 1),
                                    do_sums=(h == 0))
                # normalize + output
                if dbg and h == 0:
                    dsb = outp.tile([P, 64], F32, tag="dsb", name="dsb")
                    nc.vector.tensor_copy(dsb[:], sums_bank[:])
                    nc.gpsimd.dma_start(out=d_sums[:], in_=dsb[:])
                for t in range(NT):
                    if h == 0:
                        rc = outp.tile([P, 1], F32, tag=f"rec{t}", name=f"rec{t}")
                        if t == 0:
                            rec = {}
                        rec[t] = rc
                        nc.vector.reciprocal(rc[:], sums_bank[:, t * 16:t * 16 + 1])
                    ot = outp.tile([P, 512], F32, tag="ot", name="ot")
                    nc.vector.tensor_scalar_mul(ot[:], av[t][:], rec[t][:])
                    nc.sync.dma_start(
                        out=out[t * P:(t + 1) * P, h * 512:(h + 1) * 512], in_=ot[:])
    return nc


_CACHE = {}


def _get_nc():
    if "nc" not in _CACHE:
        nc = build_nc()
        nc.compile()
        _CACHE["nc"] = nc
    return _CACHE["nc"]


def build_in_maps(inputs):
    x_q = np.asarray(inputs["encodings_for_q"], dtype=np.float32)
    x_k = np.asarray(inputs["encodings_for_k"], dtype=np.float32)
    x_v = np.asarray(inputs["encodings_for_v"], dtype=np.float32)
    W_q = np.asarray(inputs["W_q"], dtype=np.float32)
    W_k = np.asarray(inputs["W_k"], dtype=np.float32)
    W_v = np.asarray(inputs["W_v"], dtype=np.float32)

    # no 1/sqrt(D) folding: the exp activation applies scale=1/32
    wqT = np.ascontiguousarray(W_q.T).astype(bf16)
    wkT = np.ascontiguousarray(W_k.T).astype(bf16)
    wvT = np.ascontiguousarray(W_v.T).astype(bf16)

    ik = np.arange(P)[:, None]
    iq = np.arange(P)[None, :]

    in_maps = []
    for c in range(NCORES):
        rows = np.arange(c, S, NCORES)
        keys = np.concatenate([np.arange(P * (8 * s + c), P * (8 * s + c) + P)
                               for s in range(NT)])
        masks = np.stack([(128 * d + ik <= 8 * iq + c) for d in range(8)]
                         + [(128 * c + ik <= 8 * iq + c)])  # slot 8 = own diag (d=c)
        biasv = np.zeros((P, NKT), dtype=np.float32)
        biasv[:, [c, c + 8, c + 16, c + 24]] = -1e9   # zero own-rank stream tiles
        in_maps.append(dict(
            xqT=np.ascontiguousarray(x_q[rows].T).astype(bf16),
            xkT=np.ascontiguousarray(x_k[keys].T).astype(bf16),
            xvT=np.ascontiguousarray(x_v[keys].T).astype(bf16),
            wqT=wqT, wkT=wkT, wvT=wvT,
            masks=masks.astype(bf16),
            biasv=biasv,
        ))
    return in_maps


def kernel(**inputs):
    nc = _get_nc()
    in_maps = build_in_maps(inputs)
    res = run_bass_kernel_spmd(nc, in_maps, list(range(NCORES)))
    full = np.empty((S, D), dtype=np.float32)
    for c in range(NCORES):
        full[c::NCORES] = np.asarray(res.results[c]["out"], dtype=np.float32)
    return full
